# revision 7
# baseline (speedup 1.0000x reference)
"""Multi-head self-attention (B=4, N=2048, D=1024, H=16) on 8 trn2 NeuronCores.

Sharding: 8 shards = (batch, query-half).  Core c handles batch c//2 and query
rows [(c%2)*1024, (c%2)*1024+1024).  Each core receives its batch's z with the
rows rolled so that its query rows come first; rolling permutes the key/value
sequence order, which attention output is invariant to.  K/V are computed for
the full 2048-row sequence on both cores of a batch pair (duplicated compute,
no collectives needed).

Per-core kernel (Tile):
  1. PE-transpose z -> zT (din-major), fp32.
  2. Q^T/K^T (d-major) and V (natural, with a ones column appended per head)
     projections via float32r matmuls; K^T/Q^T spilled to DRAM scratch.
  3. Per head: scores S^T = K Q^T (f32r), exp(s/8) on ACT -> bf16,
     P^T@V via matmul with V|ones (denominator accumulates in row 64),
     reciprocal + gpsimd partition-broadcast, normalized attn^T in fp32.
  4. Final projection attn @ w_o + b_o in f32r, bias via partition-broadcast.
"""

import os
import sys

_TRN_REPO = "/opt/trn_rl_repo"
if os.path.isdir(_TRN_REPO) and _TRN_REPO not in sys.path:
    sys.path.insert(0, _TRN_REPO)

import numpy as np

import concourse.bass as bass  # noqa: E402
import concourse.mybir as mybir  # noqa: E402
from concourse import bacc  # noqa: E402
from concourse.bass_utils import run_bass_kernel_spmd  # noqa: E402
from concourse.masks import make_identity  # noqa: E402
from concourse.tile import TileContext  # noqa: E402

F32 = mybir.dt.float32
F32R = mybir.dt.float32r
BF16 = mybir.dt.bfloat16
MULT = mybir.AluOpType.mult
ADD = mybir.AluOpType.add
EXP = mybir.ActivationFunctionType.Exp

N_CORES = 8
B, N, D = 4, 2048, 1024
H, HD = 16, 64
NQ = N // 2  # query rows per core
P = 128
DC = D // P  # 8 din/dout chunks of 128
NKC = N // P  # 16 key chunks of 128
SCALE = 1.0 / 8.0  # 1/sqrt(HD)


def _build():
    nc = bacc.Bacc("TRN2", target_bir_lowering=False, debug=False,
                   num_devices=N_CORES)
    z_d = nc.declare_dram_parameter("z", [N, D], F32, isOutput=False)
    wq_d = nc.declare_dram_parameter("w_q", [D, D], F32R, isOutput=False)
    wk_d = nc.declare_dram_parameter("w_k", [D, D], F32R, isOutput=False)
    wv_d = nc.declare_dram_parameter("w_v", [D, D], F32R, isOutput=False)
    wo_d = nc.declare_dram_parameter("w_o", [D, D], F32R, isOutput=False)
    bo_d = nc.declare_dram_parameter("b_o", [D], F32, isOutput=False)
    out_d = nc.declare_dram_parameter("out", [NQ, D], F32, isOutput=True)

    # DRAM scratch: K^T/Q^T in partition-major layout for clean reload.
    kts_d = nc.dram_tensor("kts", [P, DC, N], F32R)
    qts_d = nc.dram_tensor("qts", [P, DC, NQ], F32R)

    with TileContext(nc) as tc:
        with tc.tile_pool(name="const", bufs=1) as constp, \
             tc.tile_pool(name="vpool", bufs=1) as vpool:
            ident = constp.tile([P, P], F32)
            make_identity(nc, ident)
            # V' = [V_h | 1] per head: [P, key-chunk, head, 65] bf16
            vp = vpool.tile([P, NKC, H, HD + 1], BF16)
            nc.vector.memset(vp[:, :, :, HD], 1.0)

            # ---------------- Phase 1: zT + projections ----------------
            with tc.tile_pool(name="zin", bufs=1) as zinp, \
                 tc.tile_pool(name="zt", bufs=2) as ztp, \
                 tc.tile_pool(name="wt", bufs=3) as wtp, \
                 tc.tile_pool(name="stg", bufs=3) as stgp, \
                 tc.tile_pool(name="pst", bufs=2, space="PSUM") as pst, \
                 tc.tile_pool(name="psp", bufs=6, space="PSUM") as psp:

                wk_sb = wtp.tile([P, DC, D], F32R, tag="w")
                nc.sync.dma_start(wk_sb[:], wk_d.rearrange("(c p) o -> p c o", p=P))
                wv_sb = wtp.tile([P, DC, D], F32R, tag="w")
                nc.sync.dma_start(wv_sb[:], wv_d.rearrange("(c p) o -> p c o", p=P))
                wq_sb = wtp.tile([P, DC, D], F32R, tag="w")
                nc.sync.dma_start(wq_sb[:], wq_d.rearrange("(c p) o -> p c o", p=P))

                for n5 in range(N // 512):  # 4 big chunks of 512 seq rows
                    # transpose 512 z rows -> ztc [P, DC, 512]
                    ztc = ztp.tile([P, DC, 512], F32R)
                    zt_in = zinp.tile([P, 4, D], F32)
                    nc.sync.dma_start(
                        zt_in[:],
                        z_d[n5 * 512:(n5 + 1) * 512, :].rearrange(
                            "(r p) d -> p r d", p=P))
                    for dc in range(DC):
                        ps = pst.tile([P, 512], F32)
                        for r in range(4):
                            nc.tensor.transpose(
                                ps[:, r * P:(r + 1) * P],
                                zt_in[:, r, dc * P:(dc + 1) * P],
                                ident[:])
                        nc.vector.tensor_copy(ztc[:, dc, :], ps[:])

                    # K^T chunk: [dout, 512] for all 8 dout chunks
                    for og in range(2):
                        pss = [psp.tile([P, 512], F32, name="pp") for _ in range(4)]
                        for dc in range(DC):
                            for j in range(4):
                                oc = og * 4 + j
                                nc.tensor.matmul(
                                    pss[j][:],
                                    lhsT=(wk_sb[:, dc, oc * P:(oc + 1) * P]),
                                    rhs=(ztc[:, dc, :]),
                                    start=(dc == 0), stop=(dc == DC - 1))
                        for j in range(4):
                            st = stgp.tile([P, 512], F32R)
                            nc.vector.tensor_copy(st[:], pss[j][:])
                            nc.sync.dma_start(
                                kts_d[:, og * 4 + j, n5 * 512:(n5 + 1) * 512], st[:])

                    # Q^T chunk (first 1024 rows only)
                    if n5 < NQ // 512:
                        for og in range(2):
                            pss = [psp.tile([P, 512], F32, name="pp") for _ in range(4)]
                            for dc in range(DC):
                                for j in range(4):
                                    oc = og * 4 + j
                                    nc.tensor.matmul(
                                        pss[j][:],
                                        lhsT=(wq_sb[:, dc, oc * P:(oc + 1) * P]),
                                        rhs=(ztc[:, dc, :]),
                                        start=(dc == 0), stop=(dc == DC - 1))
                            for j in range(4):
                                st = stgp.tile([P, 512], F32R)
                                nc.vector.tensor_copy(st[:], pss[j][:])
                                nc.sync.dma_start(
                                    qts_d[:, og * 4 + j, n5 * 512:(n5 + 1) * 512],
                                    st[:])

                    # V chunk: natural [k, dout] -> V' (strided per head)
                    for kcp in range(2):
                        pss = [psp.tile([P, 512], F32, name="pp") for _ in range(4)]
                        for dc in range(DC):
                            for i2 in range(2):
                                kc4 = kcp * 2 + i2
                                lh = (ztc[:, dc, kc4 * P:(kc4 + 1) * P])
                                for oc2 in range(2):
                                    nc.tensor.matmul(
                                        pss[i2 * 2 + oc2][:],
                                        lhsT=lh,
                                        rhs=(wv_sb[:, dc, oc2 * 512:(oc2 + 1) * 512]),
                                        start=(dc == 0), stop=(dc == DC - 1))
                        for i2 in range(2):
                            kcg = n5 * 4 + kcp * 2 + i2
                            for oc2 in range(2):
                                nc.vector.tensor_copy(
                                    vp[:, kcg, oc2 * 8:(oc2 + 1) * 8, 0:HD],
                                    pss[i2 * 2 + oc2].rearrange(
                                        "p (h d) -> p h d", d=HD))

            # ---------------- Phases 2+3 ----------------
            with tc.tile_pool(name="at", bufs=1) as atp:
                attnT = atp.tile([P, DC, NQ], F32R)

                # Phase 2: attention per head pair
                with tc.tile_pool(name="kt", bufs=2) as ktpool, \
                     tc.tile_pool(name="qt", bufs=2) as qtpool, \
                     tc.tile_pool(name="es", bufs=8) as esp, \
                     tc.tile_pool(name="rc", bufs=4) as recp, \
                     tc.tile_pool(name="pss", bufs=4, space="PSUM") as ssp, \
                     tc.tile_pool(name="pvo", bufs=4, space="PSUM") as pvp:
                    for hc in range(H // 2):
                        ktp = ktpool.tile([P, N], F32R)
                        nc.sync.dma_start(ktp[:], kts_d[:, hc, :])
                        qtp = qtpool.tile([P, NQ], F32R)
                        nc.sync.dma_start(qtp[:], qts_d[:, hc, :])
                        for sub in range(2):
                            po = 64 * sub
                            h = 2 * hc + sub
                            pso = [pvp.tile([P, 512], F32, name="pvo") for _ in range(2)]
                            for kc in range(NKC):
                                es = esp.tile([P, NQ], BF16)
                                pss = [ssp.tile([P, 512], F32, name="pss") for _ in range(2)]
                                for qc in range(2):
                                    nc.tensor.matmul(
                                        pss[qc][:],
                                        lhsT=(ktp[po:po + 64, kc * P:(kc + 1) * P]),
                                        rhs=(qtp[po:po + 64,
                                                   qc * 512:(qc + 1) * 512]))
                                for qc in range(2):
                                    nc.scalar.activation(
                                        es[:, qc * 512:(qc + 1) * 512],
                                        pss[qc][:], EXP, scale=SCALE)
                                lh = vp[:, kc, h, :]
                                for qc in range(2):
                                    nc.tensor.matmul(
                                        pso[qc][0:HD + 1, :],
                                        lhsT=lh,
                                        rhs=es[:, qc * 512:(qc + 1) * 512],
                                        start=(kc == 0), stop=(kc == NKC - 1))
                            for qc in range(2):
                                rec = recp.tile([1, 512], F32, tag="rec")
                                nc.vector.reciprocal(rec[:], pso[qc][HD:HD + 1, :])
                                rb = recp.tile([64, 512], F32, tag="rb")
                                nc.gpsimd.partition_broadcast(rb[:], rec[:])
                                nc.vector.tensor_tensor(
                                    attnT[po:po + 64, hc, qc * 512:(qc + 1) * 512],
                                    pso[qc][0:HD, :], rb[:], MULT)

                # Phase 3: final projection + bias
                with tc.tile_pool(name="wo", bufs=1) as wop, \
                     tc.tile_pool(name="ot", bufs=4) as outp, \
                     tc.tile_pool(name="psf", bufs=4, space="PSUM") as fpp:
                    bo_sb = wop.tile([1, D], F32)
                    nc.sync.dma_start(bo_sb[:], bo_d[None, :])
                    bo_bc = wop.tile([P, D], F32)
                    nc.gpsimd.partition_broadcast(bo_bc[:], bo_sb[:])
                    wo_sb = wop.tile([P, DC, D], F32R)
                    nc.sync.dma_start(wo_sb[:], wo_d.rearrange("(c p) o -> p c o", p=P))
                    for q8 in range(NQ // P):
                        psf = [fpp.tile([P, 512], F32, name="pf") for _ in range(2)]
                        for dc in range(DC):
                            lh = (attnT[:, dc, q8 * P:(q8 + 1) * P])
                            for oc2 in range(2):
                                nc.tensor.matmul(
                                    psf[oc2][:],
                                    lhsT=lh,
                                    rhs=(wo_sb[:, dc, oc2 * 512:(oc2 + 1) * 512]),
                                    start=(dc == 0), stop=(dc == DC - 1))
                        for oc2 in range(2):
                            ot = outp.tile([P, 512], F32)
                            nc.vector.tensor_tensor(
                                ot[:], psf[oc2][:],
                                bo_bc[:, oc2 * 512:(oc2 + 1) * 512], ADD)
                            nc.sync.dma_start(
                                out_d[q8 * P:(q8 + 1) * P,
                                      oc2 * 512:(oc2 + 1) * 512], ot[:])

    nc.compile()
    return nc


_NC_CACHE = None


def _get_nc():
    global _NC_CACHE
    if _NC_CACHE is None:
        _NC_CACHE = _build()
    return _NC_CACHE


def _run(z, w_q, w_k, w_v, w_o, b_o, **spmd_kwargs):
    z = np.ascontiguousarray(np.asarray(z, dtype=np.float32))
    w_q = np.ascontiguousarray(np.asarray(w_q, dtype=np.float32))
    w_k = np.ascontiguousarray(np.asarray(w_k, dtype=np.float32))
    w_v = np.ascontiguousarray(np.asarray(w_v, dtype=np.float32))
    w_o = np.ascontiguousarray(np.asarray(w_o, dtype=np.float32))
    b_o = np.ascontiguousarray(np.asarray(b_o, dtype=np.float32))
    assert z.shape == (B, N, D)

    nc = _get_nc()
    in_maps = []
    for c in range(N_CORES):
        b = c // 2
        off = (c % 2) * NQ
        zc = np.ascontiguousarray(np.concatenate([z[b, off:], z[b, :off]], axis=0))
        in_maps.append({"z": zc, "w_q": w_q, "w_k": w_k, "w_v": w_v,
                        "w_o": w_o, "b_o": b_o})

    res = run_bass_kernel_spmd(nc, in_maps, core_ids=list(range(N_CORES)),
                               **spmd_kwargs)
    out = np.empty((B, N, D), dtype=np.float32)
    for c in range(N_CORES):
        b = c // 2
        off = (c % 2) * NQ
        out[b, off:off + NQ, :] = res.results[c]["out"]
    return out, res


def kernel(z, w_q, w_k, w_v, w_o, b_o):
    out, _ = _run(z, w_q, w_k, w_v, w_o, b_o)
    return out
